# revision 24
# baseline (speedup 1.0000x reference)
"""GMM log-likelihood kernel for Trainium2 (Bass/Tile), 8-core data-parallel.

Math (host precompute in f64):
  B' = L^{-1} / sqrt(2),  S_k(x) = ||B'_k x||^2 = 0.5 maha-quadratic part
  wlp_k(x) = -S_k(x) + w_k . x + (C_k - m0),  w_k = B^T B mu_k,
  C_k = log pi_k - d/2 log 2pi - half_logdet_k - 0.5 ||B mu_k||^2
  out = sum_x [ m0 + log sum_k exp(wlp_k(x)) ]

Per core (25000 samples, padded to 196 tiles of 128):
  The PE runs in 64x128 row-tiled mode: even data-tiles' x^T lives in SBUF
  partitions 0:64 and computes on array rows 0:64 (tile T0), odd tiles in
  partitions 64:128 on rows 64:128 (T8) -- the two matmul streams execute
  CONCURRENTLY (the 66-row contraction only half-fills the array, and the
  HAM clock stays at 1.2 GHz for this duty cycle, so packing two matmuls
  recovers the lost 2x).  Per tile: Y psum [128,1024] (2 banks, 3 bufs)
  + a 16-col lin matmul into a per-parity group bank; a DVE STT folds
  lin+const to SBUF.  Squares split between ACT (Square -> bf16) and a
  custom DVE op sq(a)+b fusing the 64->32 fold while reading PSUM; a bf16
  TT tree (2x DVE mode) does 32->2 per (tile,k) batched per 14-tile group.
  Phase 2: exp / k-reduce / ln / mask / reduce / gpsimd partition-fold.
Host sums the 8 per-core scalars (+ m0 per real sample).
"""

import numpy as np

N_COMPONENTS = 16
N_FEATURES = 64
N_SAMPLES = 200000
N_CORES = 8
PER_CORE = N_SAMPLES // N_CORES          # 25000
TILE_P = 128
N_TILES = -(-PER_CORE // TILE_P)         # 196 (ceil)
PADDED = N_TILES * TILE_P                # 25088
KD = N_COMPONENTS * N_FEATURES           # 1024
K = N_COMPONENTS

GROUP_TILES = 14                         # tiles per tree batch
N_GROUPS = N_TILES // GROUP_TILES        # 14
PAIRS_PER_GROUP = GROUP_TILES // 2       # 7
CHUNK_PAIRS = 14                         # DMA chunk = 2 groups
N_CHUNKS = N_TILES // (2 * CHUNK_PAIRS)  # 7

# pair index -> drain path: "H" pairs split each tile's square between ACT
# (first half of each k-group) and the DVE custom op; "A" pairs let ACT
# square everything and GPSIMD fold both tiles in one batched TT (three-way
# ACT/DVE/GPSIMD busy-balance).
PAIR_PERIOD = 15
H_PAIR_MOD = (0, 2, 4, 6, 8, 10, 12)

_CACHE = {}


def _register_sq2():
    """Custom DVE op  out = sq(in0) + in1  (f32 internal): in0 = raw-Y half
    from PSUM (DVE may read only ONE PSUM operand), in1 = the ACT-squared
    other half from SBUF."""
    from concourse import dve_ops
    from concourse.dve_spec import Spec, Src0, Src1, sq, lower, _has_src1
    from concourse.dve_uop import DveOpSpec

    if any(op.name == "SQ1_ADD_ANT" for op in dve_ops.OPS):
        return next(op for op in dve_ops.OPS if op.name == "SQ1_ADD_ANT")
    spec = Spec(
        body=sq(Src0) + Src1,
        reference=lambda in0, in1, s0, s1, imm2: (
            in0.astype(np.float32) ** 2 + in1.astype(np.float32)),
    )
    shas = {}
    for ver in ("v3", "v4"):
        c = DveOpSpec(name="SQ1_ADD_ANT", opcode=17,
                      uops=lower(spec, ver=ver), rd1_en=_has_src1(spec))
        shas[ver] = c.sha(ver)
    op = dve_ops.DveOp("SQ1_ADD_ANT", spec, subdim=False, uops_sha=shas)
    row = max(dve_ops._SUB_OPCODE_FOR_NAME.values()) + 1
    assert row < 0x20
    dve_ops.OPS.append(op)
    dve_ops._SUB_OPCODE_FOR_NAME[op.name] = row
    dve_ops.CUSTOM_DVE_SPECS[op.name] = spec
    return op


def _build_nc():
    import concourse.tile as tile
    from concourse import bacc, mybir, bass_isa

    sq2 = _register_sq2()

    f32 = mybir.dt.float32
    bf16 = mybir.dt.bfloat16
    W = GROUP_TILES * K                  # 224 wlp columns per group
    HALF = N_TILES * TILE_P // 2         # 12544 columns of paired x^T

    nc = bacc.Bacc("TRN2", target_bir_lowering=False, debug=False,
                   num_devices=N_CORES)

    xp = nc.dram_tensor("xp", [128, HALF], bf16, kind="ExternalInput").ap()
    bm = nc.dram_tensor("bm", [128, KD + K], bf16, kind="ExternalInput").ap()
    lconst = nc.dram_tensor("lconst", [128, W], f32, kind="ExternalInput").ap()
    mask = nc.dram_tensor("mask", [128, N_TILES], f32, kind="ExternalInput").ap()
    out = nc.dram_tensor("out", [1, 1], f32, kind="ExternalOutput").ap()

    with tile.TileContext(nc) as tc:
        with (
            tc.tile_pool(name="const", bufs=1) as const_pool,
            tc.tile_pool(name="xin", bufs=2) as xin_pool,
            tc.tile_pool(name="ysq", bufs=4) as ysq_pool,
            tc.tile_pool(name="sq32", bufs=2) as sq32_pool,
            tc.tile_pool(name="tree", bufs=2) as tree_pool,
            tc.tile_pool(name="lin", bufs=2) as lin_pool,
            tc.tile_pool(name="wb", bufs=1) as wb_pool,
            tc.tile_pool(name="yp", bufs=3, space="PSUM") as yp_pool,
            tc.tile_pool(name="lpe", bufs=1, space="PSUM") as lpe_pool,
            tc.tile_pool(name="lpo", bufs=1, space="PSUM") as lpo_pool,
        ):
            bms = const_pool.tile([128, KD + K], bf16)
            nc.sync.dma_start(bms[:], bm[:])
            lcs = const_pool.tile([128, W], f32)
            msks = const_pool.tile([128, N_TILES], f32)

            wlp = wb_pool.tile([128, N_TILES * K], f32)
            ebuf = wb_pool.tile([128, N_TILES * K], f32)
            rsum = wb_pool.tile([128, N_TILES], f32)

            def drain_h(yp, sq32, sqc):
                """H path: ACT squares the 0:32 halves, DVE custom squares
                32:64 from PSUM and adds, emitting sq32[:, sqc:sqc+512]."""
                ypv = yp[:].rearrange("p (k i) -> p k i", i=64)
                ysa = ysq_pool.tile([128, 512], bf16, tag="ysa")
                nc.scalar.activation(
                    ysa[:].rearrange("p (k i) -> p k i", i=32),
                    ypv[:, :, 0:32],
                    mybir.ActivationFunctionType.Square)
                nc.vector._custom_dve(
                    sq2,
                    out=sq32[:, sqc:sqc + 512]
                    .rearrange("p (k i) -> p k i", i=32),
                    in0=ypv[:, :, 32:64],
                    in1=ysa[:].rearrange("p (k i) -> p k i", i=32),
                )

            def drain_a(ype, ypo, sq32, sqc):
                """A path: ACT squares both tiles -> one [128,2048] bf16
                buffer; GPSIMD (otherwise idle) folds 64->32 for the pair."""
                ysq = ysq_pool.tile([128, 2 * KD], bf16, tag="ysq")
                nc.scalar.activation(
                    ysq[:, 0:KD], ype[:], mybir.ActivationFunctionType.Square)
                nc.scalar.activation(
                    ysq[:, KD:2 * KD], ypo[:],
                    mybir.ActivationFunctionType.Square)
                yv = ysq[:].rearrange("p (k i) -> p k i", i=64)
                nc.vector.tensor_add(
                    sq32[:, sqc:sqc + 1024]
                    .rearrange("p (k i) -> p k i", i=32),
                    yv[:, :, 0:32], yv[:, :, 32:64])

            def make_tree(g, sq32, linb):
                """Deferred emitter: tree 32 -> 2 (bf16 TT @2x), STTs,
                then this group's exp + k-reduce (keeps the end tail short;
                Exp shares the ACT table set with Square, so no reloads)."""
                def emit():
                    t16 = tree_pool.tile([128, W * 16], bf16, tag="t16")
                    v = sq32[:].rearrange("p (w i) -> p w i", i=32)
                    nc.vector.tensor_add(
                        t16[:].rearrange("p (w i) -> p w i", i=16),
                        v[:, :, 0:16], v[:, :, 16:32])
                    t8 = tree_pool.tile([128, W * 8], bf16, tag="t8")
                    v = t16[:].rearrange("p (w i) -> p w i", i=16)
                    nc.vector.tensor_add(
                        t8[:].rearrange("p (w i) -> p w i", i=8),
                        v[:, :, 0:8], v[:, :, 8:16])
                    t4 = tree_pool.tile([128, W * 4], bf16, tag="t4")
                    v = t8[:].rearrange("p (w i) -> p w i", i=8)
                    nc.vector.tensor_add(
                        t4[:].rearrange("p (w i) -> p w i", i=4),
                        v[:, :, 0:4], v[:, :, 4:8])
                    t2 = tree_pool.tile([128, W * 2], bf16, tag="t2")
                    v = t4[:].rearrange("p (w i) -> p w i", i=4)
                    nc.vector.tensor_add(
                        t2[:].rearrange("p (w i) -> p w i", i=2),
                        v[:, :, 0:2], v[:, :, 2:4])
                    s1 = tree_pool.tile([128, W], f32, tag="s1")
                    v = t2[:].rearrange("p (w i) -> p w i", i=2)
                    nc.vector.scalar_tensor_tensor(
                        s1[:], v[:, :, 0:1], -1.0, v[:, :, 1:2],
                        op0=mybir.AluOpType.mult,
                        op1=mybir.AluOpType.subtract)
                    nc.vector.scalar_tensor_tensor(
                        wlp[:, g * W:(g + 1) * W], s1[:], 1.0, linb[:],
                        op0=mybir.AluOpType.mult, op1=mybir.AluOpType.add)
                    nc.scalar.activation(
                        ebuf[:, g * W:(g + 1) * W], wlp[:, g * W:(g + 1) * W],
                        mybir.ActivationFunctionType.Exp)
                    nc.vector.reduce_sum(
                        rsum[:, g * GROUP_TILES:(g + 1) * GROUP_TILES],
                        ebuf[:, g * W:(g + 1) * W]
                        .rearrange("p (t k) -> p t k", k=K),
                        axis=mybir.AxisListType.X)
                return emit

            pending_tree = None
            for c in range(N_CHUNKS):
                xb = xin_pool.tile([128, CHUNK_PAIRS * TILE_P], bf16, tag="xb")
                c0 = c * CHUNK_PAIRS * TILE_P
                nc.sync.dma_start(xb[:], xp[:, c0:c0 + CHUNK_PAIRS * TILE_P])
                if c == 0:
                    # late-needed constants AFTER the first data chunk, so the
                    # first matmuls aren't queued behind them on the DMA ring
                    nc.sync.dma_start(lcs[:], lconst[:])
                    nc.sync.dma_start(msks[:], mask[:])

                for gl in range(2):              # two 14-tile groups per chunk
                    g = 2 * c + gl
                    sq32 = sq32_pool.tile([128, GROUP_TILES * 512], bf16,
                                          tag="sq32")
                    lpe = lpe_pool.tile([128, PAIRS_PER_GROUP * K], f32,
                                        tag="lpe")
                    lpo = lpo_pool.tile([128, PAIRS_PER_GROUP * K], f32,
                                        tag="lpo")
                    for pl in range(PAIRS_PER_GROUP):
                        xc = (gl * PAIRS_PER_GROUP + pl) * TILE_P
                        ype = yp_pool.tile([128, KD], f32, tag="yp")
                        ypo = yp_pool.tile([128, KD], f32, tag="yp")
                        lhsE = xb[0:64, xc:xc + TILE_P]
                        lhsO = xb[64:128, xc:xc + TILE_P]
                        nc.tensor.matmul(ype[:, 0:512], lhsE,
                                         bms[0:64, 0:512])
                        nc.tensor.matmul(ypo[:, 0:512], lhsO,
                                         bms[64:128, 0:512])
                        nc.tensor.matmul(ype[:, 512:1024], lhsE,
                                         bms[0:64, 512:1024])
                        nc.tensor.matmul(ypo[:, 512:1024], lhsO,
                                         bms[64:128, 512:1024])
                        nc.tensor.matmul(lpe[:, pl * K:(pl + 1) * K], lhsE,
                                         bms[0:64, KD:KD + K])
                        nc.tensor.matmul(lpo[:, pl * K:(pl + 1) * K], lhsO,
                                         bms[64:128, KD:KD + K])
                        pair = g * PAIRS_PER_GROUP + pl
                        if pair % PAIR_PERIOD in H_PAIR_MOD:
                            drain_h(ype, sq32, (2 * pl) * 512)
                            drain_h(ypo, sq32, (2 * pl + 1) * 512)
                        else:
                            drain_a(ype, ypo, sq32, (2 * pl) * 512)
                        if pl == 1 and pending_tree is not None:
                            # previous group's tree, emitted mid-stream so it
                            # doesn't head-block this group's per-tile DVE ops
                            pending_tree()
                            pending_tree = None

                    # lin + const -> SBUF (frees the lp banks early)
                    linb = lin_pool.tile([128, W], f32, tag="linb")
                    lbv = linb[:].rearrange("p (t k) -> p t k", k=K)
                    lcv = lcs[:].rearrange("p (t k) -> p t k", k=K)
                    nc.vector.scalar_tensor_tensor(
                        lbv[:, 0:GROUP_TILES:2, :],
                        lpe[:].rearrange("p (t k) -> p t k", k=K),
                        1.0, lcv[:, 0:GROUP_TILES:2, :],
                        op0=mybir.AluOpType.mult, op1=mybir.AluOpType.add)
                    nc.vector.scalar_tensor_tensor(
                        lbv[:, 1:GROUP_TILES:2, :],
                        lpo[:].rearrange("p (t k) -> p t k", k=K),
                        1.0, lcv[:, 1:GROUP_TILES:2, :],
                        op0=mybir.AluOpType.mult, op1=mybir.AluOpType.add)
                    pending_tree = make_tree(g, sq32, linb)
            pending_tree()

            # phase 2 (exp/k-reduce already done per group)
            lnr = const_pool.tile([128, N_TILES], f32)
            nc.scalar.activation(lnr[:], rsum[:],
                                 mybir.ActivationFunctionType.Ln)
            msum = const_pool.tile([128, N_TILES], f32)
            nc.vector.tensor_mul(msum[:], lnr[:], msks[:])
            csum = const_pool.tile([128, 1], f32)
            nc.vector.reduce_sum(csum[:], msum[:], axis=mybir.AxisListType.X)

            res = const_pool.tile([128, 1], f32)
            nc.gpsimd.partition_all_reduce(res[:], csum[:], channels=128,
                                           reduce_op=bass_isa.ReduceOp.add)
            nc.sync.dma_start(out[:], res[0:1, :])

    nc.compile()
    return nc


def _precompute(weights, means, covariances):
    """Host-side O(K d^3) prep in float64. Returns (bm, lconst_row, m0)."""
    import ml_dtypes

    Kc, d = means.shape
    L = np.linalg.cholesky(covariances.astype(np.float64))
    half_logdet = np.log(np.diagonal(L, axis1=-2, axis2=-1)).sum(-1)
    eye = np.eye(d)
    B = np.stack([np.linalg.solve(L[k], eye) for k in range(Kc)])  # L^-1
    mu = means.astype(np.float64)
    c = np.einsum('kij,kj->ki', B, mu)                # B mu
    w_lin = np.einsum('kij,ki->kj', B, c)             # B^T B mu
    r = (c * c).sum(-1)
    C = (np.log(weights.astype(np.float64))
         - 0.5 * d * np.log(2.0 * np.pi) - half_logdet - 0.5 * r)
    m0 = float(C.max()) - 20.0
    Bs = B / np.sqrt(2.0)                             # S = 0.5 ||B x||^2

    half = np.zeros((d, KD + Kc), np.float32)
    for k in range(Kc):
        half[:, k * d:(k + 1) * d] = Bs[k].T.astype(np.float32)
    half[:, KD:] = w_lin.T.astype(np.float32)
    bm = np.vstack([half, half]).astype(ml_dtypes.bfloat16)  # [128, 1040]
    lconst_row = (C - m0).astype(np.float32)                 # [16]
    return bm, lconst_row, m0


def _make_inputs(data, bm, lconst_row):
    """8 per-core input maps: x^T parity-split into top/bottom partitions."""
    import ml_dtypes

    lconst = np.tile(lconst_row, GROUP_TILES)[None, :].repeat(128, 0)
    lconst = np.ascontiguousarray(lconst, np.float32)
    mask = np.zeros((128, N_TILES), np.float32)
    for t in range(N_TILES):
        v = min(max(PER_CORE - t * TILE_P, 0), TILE_P)
        mask[:v, t] = 1.0

    in_maps = []
    for cc in range(N_CORES):
        sl = data[cc * PER_CORE:(cc + 1) * PER_CORE]
        xt = np.zeros((N_FEATURES, PADDED), np.float32)
        xt[:, :PER_CORE] = sl.T
        xt = xt.reshape(N_FEATURES, N_TILES // 2, 2, TILE_P)
        xpc = np.empty((128, PADDED // 2), np.float32)
        xpc[0:64] = xt[:, :, 0, :].reshape(N_FEATURES, -1)
        xpc[64:128] = xt[:, :, 1, :].reshape(N_FEATURES, -1)
        in_maps.append({"xp": xpc.astype(ml_dtypes.bfloat16), "bm": bm,
                        "lconst": lconst, "mask": mask})
    return in_maps


def _run(data, weights, means, covariances, trace=False):
    from concourse.bass_utils import run_bass_kernel_spmd

    data = np.asarray(data, np.float32)
    bm, lconst_row, m0 = _precompute(np.asarray(weights), np.asarray(means),
                                     np.asarray(covariances))
    if "nc" not in _CACHE:
        _CACHE["nc"] = _build_nc()
    nc = _CACHE["nc"]

    in_maps = _make_inputs(data, bm, lconst_row)
    res = run_bass_kernel_spmd(nc, in_maps, list(range(N_CORES)), trace=trace)
    total = 0.0
    for cc in range(N_CORES):
        total += float(res.results[cc]["out"][0, 0]) + PER_CORE * m0
    return np.float32(total), res


def kernel(data, weights, means, covariances):
    return _run(data, weights, means, covariances)[0]


# revision 28
# speedup vs baseline: 1.1437x; 1.1437x over previous
"""GMM log-likelihood kernel for Trainium2 (Bass/Tile), 8-core data-parallel.

Math (host precompute in f64):
  B' = L^{-1} / sqrt(2),  S_k(x) = ||B'_k x||^2 = 0.5 maha-quadratic part
  wlp_k(x) = -S_k(x) + w_k . x + (C_k - m0),  w_k = B^T B mu_k,
  C_k = log pi_k - d/2 log 2pi - half_logdet_k - 0.5 ||B mu_k||^2
  out = sum_x [ m0 + log sum_k exp(wlp_k(x)) ]

Per core (25000 samples, padded to 196 tiles of 128):
  The PE runs in 64x128 row-tiled mode: even data-tiles' x^T lives in SBUF
  partitions 0:64 and computes on array rows 0:64 (tile T0), odd tiles in
  partitions 64:128 on rows 64:128 (T8) -- the two matmul streams execute
  CONCURRENTLY (the 66-row contraction only half-fills the array, and the
  HAM clock stays at 1.2 GHz for this duty cycle, so packing two matmuls
  recovers the lost 2x).  Per tile: Y psum [128,1024] (2 banks, 3 bufs)
  + a 16-col lin matmul into a per-parity group bank; a DVE STT folds
  lin+const to SBUF.  Squares split between ACT (Square -> bf16) and a
  custom DVE op sq(a)+b fusing the 64->32 fold while reading PSUM; a bf16
  TT tree (2x DVE mode) does 32->2 per (tile,k) batched per 14-tile group.
  Phase 2: exp / k-reduce / ln / mask / reduce / gpsimd partition-fold.
Host sums the 8 per-core scalars (+ m0 per real sample).
"""

import numpy as np

N_COMPONENTS = 16
N_FEATURES = 64
N_SAMPLES = 200000
N_CORES = 8
PER_CORE = N_SAMPLES // N_CORES          # 25000
TILE_P = 128
N_TILES = -(-PER_CORE // TILE_P)         # 196 (ceil)
PADDED = N_TILES * TILE_P                # 25088
KD = N_COMPONENTS * N_FEATURES           # 1024
K = N_COMPONENTS

GROUP_TILES = 14                         # tiles per tree batch
N_GROUPS = N_TILES // GROUP_TILES        # 14
PAIRS_PER_GROUP = GROUP_TILES // 2       # 7
CHUNK_PAIRS = 14                         # DMA chunk = 2 groups
N_CHUNKS = N_TILES // (2 * CHUNK_PAIRS)  # 7

# pair index -> drain path: "H" pairs split each tile's square between ACT
# (first half of each k-group) and the DVE custom op; "A" pairs let ACT
# square everything and DVE fold both tiles in one batched 2x-mode TT.
# (ACT/DVE busy-balance knob.  GPSIMD folding was tried and regressed:
# ~4.5us/fold on Q7 + shared-SBUF-port contention with DVE.)
PAIR_PERIOD = 15
H_PAIR_MOD = (0, 2, 4, 6, 8, 10)

_CACHE = {}


def _register_sq2():
    """Custom DVE op  out = sq(in0) + in1  (f32 internal): in0 = raw-Y half
    from PSUM (DVE may read only ONE PSUM operand), in1 = the ACT-squared
    other half from SBUF."""
    from concourse import dve_ops
    from concourse.dve_spec import Spec, Src0, Src1, sq, lower, _has_src1
    from concourse.dve_uop import DveOpSpec

    if any(op.name == "SQ1_ADD_ANT" for op in dve_ops.OPS):
        return next(op for op in dve_ops.OPS if op.name == "SQ1_ADD_ANT")
    spec = Spec(
        body=sq(Src0) + Src1,
        reference=lambda in0, in1, s0, s1, imm2: (
            in0.astype(np.float32) ** 2 + in1.astype(np.float32)),
    )
    shas = {}
    for ver in ("v3", "v4"):
        c = DveOpSpec(name="SQ1_ADD_ANT", opcode=17,
                      uops=lower(spec, ver=ver), rd1_en=_has_src1(spec))
        shas[ver] = c.sha(ver)
    op = dve_ops.DveOp("SQ1_ADD_ANT", spec, subdim=False, uops_sha=shas)
    row = max(dve_ops._SUB_OPCODE_FOR_NAME.values()) + 1
    assert row < 0x20
    dve_ops.OPS.append(op)
    dve_ops._SUB_OPCODE_FOR_NAME[op.name] = row
    dve_ops.CUSTOM_DVE_SPECS[op.name] = spec
    return op


def _build_nc():
    import concourse.tile as tile
    from concourse import bacc, mybir, bass_isa

    sq2 = _register_sq2()

    f32 = mybir.dt.float32
    bf16 = mybir.dt.bfloat16
    W = GROUP_TILES * K                  # 224 wlp columns per group
    HALF = N_TILES * TILE_P // 2         # 12544 columns of paired x^T

    nc = bacc.Bacc("TRN2", target_bir_lowering=False, debug=False,
                   num_devices=N_CORES)

    xp = nc.dram_tensor("xp", [128, HALF], bf16, kind="ExternalInput").ap()
    bm = nc.dram_tensor("bm", [128, KD + K], bf16, kind="ExternalInput").ap()
    lconst = nc.dram_tensor("lconst", [128, W], f32, kind="ExternalInput").ap()
    mask = nc.dram_tensor("mask", [128, N_TILES], f32, kind="ExternalInput").ap()
    out = nc.dram_tensor("out", [1, 1], f32, kind="ExternalOutput").ap()

    with tile.TileContext(nc) as tc:
        with (
            tc.tile_pool(name="const", bufs=1) as const_pool,
            tc.tile_pool(name="xin", bufs=2) as xin_pool,
            tc.tile_pool(name="ysq", bufs=6) as ysq_pool,
            tc.tile_pool(name="sq32", bufs=3) as sq32_pool,
            tc.tile_pool(name="tree", bufs=2) as tree_pool,
            tc.tile_pool(name="lin", bufs=2) as lin_pool,
            tc.tile_pool(name="wb", bufs=1) as wb_pool,
            tc.tile_pool(name="yp", bufs=3, space="PSUM") as yp_pool,
            tc.tile_pool(name="lpe", bufs=1, space="PSUM") as lpe_pool,
            tc.tile_pool(name="lpo", bufs=1, space="PSUM") as lpo_pool,
        ):
            bms = const_pool.tile([128, KD + K], bf16)
            nc.sync.dma_start(bms[:], bm[:])
            lcs = const_pool.tile([128, W], f32)
            msks = const_pool.tile([128, N_TILES], f32)

            wlp = wb_pool.tile([128, N_TILES * K], f32)
            ebuf = wb_pool.tile([128, N_TILES * K], f32)
            rsum = wb_pool.tile([128, N_TILES], f32)

            def drain_h(yp, sq32, sqc):
                """H path: ACT squares the 0:32 halves, DVE custom squares
                32:64 from PSUM and adds, emitting sq32[:, sqc:sqc+512]."""
                ypv = yp[:].rearrange("p (k i) -> p k i", i=64)
                ysa = ysq_pool.tile([128, 512], bf16, tag="ysa")
                nc.scalar.activation(
                    ysa[:].rearrange("p (k i) -> p k i", i=32),
                    ypv[:, :, 0:32],
                    mybir.ActivationFunctionType.Square)
                nc.vector._custom_dve(
                    sq2,
                    out=sq32[:, sqc:sqc + 512]
                    .rearrange("p (k i) -> p k i", i=32),
                    in0=ypv[:, :, 32:64],
                    in1=ysa[:].rearrange("p (k i) -> p k i", i=32),
                )

            def drain_a(ype, ypo, sq32, sqc):
                """A path: ACT squares both tiles -> one [128,2048] bf16
                buffer; one 2x-mode DVE TT folds 64->32 for the pair."""
                ysq = ysq_pool.tile([128, 2 * KD], bf16, tag="ysq")
                nc.scalar.activation(
                    ysq[:, 0:KD], ype[:], mybir.ActivationFunctionType.Square)
                nc.scalar.activation(
                    ysq[:, KD:2 * KD], ypo[:],
                    mybir.ActivationFunctionType.Square)
                yv = ysq[:].rearrange("p (k i) -> p k i", i=64)
                nc.vector.tensor_add(
                    sq32[:, sqc:sqc + 1024]
                    .rearrange("p (k i) -> p k i", i=32),
                    yv[:, :, 0:32], yv[:, :, 32:64])

            def make_tree(g, sq32, linb):
                """Deferred emitter: tree 32 -> 2 (bf16 TT @2x), STTs,
                then this group's exp + k-reduce (keeps the end tail short;
                Exp shares the ACT table set with Square, so no reloads)."""
                def emit():
                    t16 = tree_pool.tile([128, W * 16], bf16, tag="t16")
                    v = sq32[:].rearrange("p (w i) -> p w i", i=32)
                    nc.vector.tensor_add(
                        t16[:].rearrange("p (w i) -> p w i", i=16),
                        v[:, :, 0:16], v[:, :, 16:32])
                    t8 = tree_pool.tile([128, W * 8], bf16, tag="t8")
                    v = t16[:].rearrange("p (w i) -> p w i", i=16)
                    nc.vector.tensor_add(
                        t8[:].rearrange("p (w i) -> p w i", i=8),
                        v[:, :, 0:8], v[:, :, 8:16])
                    t4 = tree_pool.tile([128, W * 4], bf16, tag="t4")
                    v = t8[:].rearrange("p (w i) -> p w i", i=8)
                    nc.vector.tensor_add(
                        t4[:].rearrange("p (w i) -> p w i", i=4),
                        v[:, :, 0:4], v[:, :, 4:8])
                    t2 = tree_pool.tile([128, W * 2], bf16, tag="t2")
                    v = t4[:].rearrange("p (w i) -> p w i", i=4)
                    nc.vector.tensor_add(
                        t2[:].rearrange("p (w i) -> p w i", i=2),
                        v[:, :, 0:2], v[:, :, 2:4])
                    s1 = tree_pool.tile([128, W], f32, tag="s1")
                    v = t2[:].rearrange("p (w i) -> p w i", i=2)
                    nc.vector.scalar_tensor_tensor(
                        s1[:], v[:, :, 0:1], -1.0, v[:, :, 1:2],
                        op0=mybir.AluOpType.mult,
                        op1=mybir.AluOpType.subtract)
                    nc.vector.scalar_tensor_tensor(
                        wlp[:, g * W:(g + 1) * W], s1[:], 1.0, linb[:],
                        op0=mybir.AluOpType.mult, op1=mybir.AluOpType.add)
                    nc.scalar.activation(
                        ebuf[:, g * W:(g + 1) * W], wlp[:, g * W:(g + 1) * W],
                        mybir.ActivationFunctionType.Exp)
                    nc.vector.reduce_sum(
                        rsum[:, g * GROUP_TILES:(g + 1) * GROUP_TILES],
                        ebuf[:, g * W:(g + 1) * W]
                        .rearrange("p (t k) -> p t k", k=K),
                        axis=mybir.AxisListType.X)
                return emit

            pending_tree = None
            for c in range(N_CHUNKS):
                xb = xin_pool.tile([128, CHUNK_PAIRS * TILE_P], bf16, tag="xb")
                c0 = c * CHUNK_PAIRS * TILE_P
                nc.sync.dma_start(xb[:], xp[:, c0:c0 + CHUNK_PAIRS * TILE_P])
                if c == 0:
                    # late-needed constants AFTER the first data chunk, so the
                    # first matmuls aren't queued behind them on the DMA ring
                    nc.sync.dma_start(lcs[:], lconst[:])
                    nc.sync.dma_start(msks[:], mask[:])

                for gl in range(2):              # two 14-tile groups per chunk
                    g = 2 * c + gl
                    sq32 = sq32_pool.tile([128, GROUP_TILES * 512], bf16,
                                          tag="sq32")
                    lpe = lpe_pool.tile([128, PAIRS_PER_GROUP * K], f32,
                                        tag="lpe")
                    lpo = lpo_pool.tile([128, PAIRS_PER_GROUP * K], f32,
                                        tag="lpo")
                    for pl in range(PAIRS_PER_GROUP):
                        xc = (gl * PAIRS_PER_GROUP + pl) * TILE_P
                        ype = yp_pool.tile([128, KD], f32, tag="yp")
                        ypo = yp_pool.tile([128, KD], f32, tag="yp")
                        lhsE = xb[0:64, xc:xc + TILE_P]
                        lhsO = xb[64:128, xc:xc + TILE_P]
                        nc.tensor.matmul(ype[:, 0:512], lhsE,
                                         bms[0:64, 0:512])
                        nc.tensor.matmul(ypo[:, 0:512], lhsO,
                                         bms[64:128, 0:512])
                        nc.tensor.matmul(ype[:, 512:1024], lhsE,
                                         bms[0:64, 512:1024])
                        nc.tensor.matmul(ypo[:, 512:1024], lhsO,
                                         bms[64:128, 512:1024])
                        nc.tensor.matmul(lpe[:, pl * K:(pl + 1) * K], lhsE,
                                         bms[0:64, KD:KD + K])
                        nc.tensor.matmul(lpo[:, pl * K:(pl + 1) * K], lhsO,
                                         bms[64:128, KD:KD + K])
                        if pl == PAIRS_PER_GROUP - 1:
                            # lin + const -> SBUF right after the last lin
                            # matmul (ahead of this pair's drains in the DVE
                            # queue) so the lp banks free before group g+1
                            # needs them
                            linb = lin_pool.tile([128, W], f32, tag="linb")
                            lbv = linb[:].rearrange("p (t k) -> p t k", k=K)
                            lcv = lcs[:].rearrange("p (t k) -> p t k", k=K)
                            nc.vector.scalar_tensor_tensor(
                                lbv[:, 0:GROUP_TILES:2, :],
                                lpe[:].rearrange("p (t k) -> p t k", k=K),
                                1.0, lcv[:, 0:GROUP_TILES:2, :],
                                op0=mybir.AluOpType.mult,
                                op1=mybir.AluOpType.add)
                            nc.vector.scalar_tensor_tensor(
                                lbv[:, 1:GROUP_TILES:2, :],
                                lpo[:].rearrange("p (t k) -> p t k", k=K),
                                1.0, lcv[:, 1:GROUP_TILES:2, :],
                                op0=mybir.AluOpType.mult,
                                op1=mybir.AluOpType.add)
                        pair = g * PAIRS_PER_GROUP + pl
                        if pair % PAIR_PERIOD in H_PAIR_MOD:
                            drain_h(ype, sq32, (2 * pl) * 512)
                            drain_h(ypo, sq32, (2 * pl + 1) * 512)
                        else:
                            drain_a(ype, ypo, sq32, (2 * pl) * 512)
                        if pl == 1 and pending_tree is not None:
                            # previous group's tree, emitted mid-stream so it
                            # doesn't head-block this group's per-tile DVE ops
                            pending_tree()
                            pending_tree = None

                    pending_tree = make_tree(g, sq32, linb)
            pending_tree()

            # phase 2 (exp/k-reduce already done per group)
            lnr = const_pool.tile([128, N_TILES], f32)
            nc.scalar.activation(lnr[:], rsum[:],
                                 mybir.ActivationFunctionType.Ln)
            msum = const_pool.tile([128, N_TILES], f32)
            nc.vector.tensor_mul(msum[:], lnr[:], msks[:])
            csum = const_pool.tile([128, 1], f32)
            nc.vector.reduce_sum(csum[:], msum[:], axis=mybir.AxisListType.X)

            res = const_pool.tile([128, 1], f32)
            nc.gpsimd.partition_all_reduce(res[:], csum[:], channels=128,
                                           reduce_op=bass_isa.ReduceOp.add)
            nc.sync.dma_start(out[:], res[0:1, :])

    nc.compile()
    return nc


def _precompute(weights, means, covariances):
    """Host-side O(K d^3) prep in float64. Returns (bm, lconst_row, m0)."""
    import ml_dtypes

    Kc, d = means.shape
    L = np.linalg.cholesky(covariances.astype(np.float64))
    half_logdet = np.log(np.diagonal(L, axis1=-2, axis2=-1)).sum(-1)
    eye = np.eye(d)
    B = np.stack([np.linalg.solve(L[k], eye) for k in range(Kc)])  # L^-1
    mu = means.astype(np.float64)
    c = np.einsum('kij,kj->ki', B, mu)                # B mu
    w_lin = np.einsum('kij,ki->kj', B, c)             # B^T B mu
    r = (c * c).sum(-1)
    C = (np.log(weights.astype(np.float64))
         - 0.5 * d * np.log(2.0 * np.pi) - half_logdet - 0.5 * r)
    m0 = float(C.max()) - 20.0
    Bs = B / np.sqrt(2.0)                             # S = 0.5 ||B x||^2

    half = np.zeros((d, KD + Kc), np.float32)
    for k in range(Kc):
        half[:, k * d:(k + 1) * d] = Bs[k].T.astype(np.float32)
    half[:, KD:] = w_lin.T.astype(np.float32)
    bm = np.vstack([half, half]).astype(ml_dtypes.bfloat16)  # [128, 1040]
    lconst_row = (C - m0).astype(np.float32)                 # [16]
    return bm, lconst_row, m0


def _make_inputs(data, bm, lconst_row):
    """8 per-core input maps: x^T parity-split into top/bottom partitions."""
    import ml_dtypes

    lconst = np.tile(lconst_row, GROUP_TILES)[None, :].repeat(128, 0)
    lconst = np.ascontiguousarray(lconst, np.float32)
    mask = np.zeros((128, N_TILES), np.float32)
    for t in range(N_TILES):
        v = min(max(PER_CORE - t * TILE_P, 0), TILE_P)
        mask[:v, t] = 1.0

    in_maps = []
    for cc in range(N_CORES):
        sl = data[cc * PER_CORE:(cc + 1) * PER_CORE]
        xt = np.zeros((N_FEATURES, PADDED), np.float32)
        xt[:, :PER_CORE] = sl.T
        xt = xt.reshape(N_FEATURES, N_TILES // 2, 2, TILE_P)
        xpc = np.empty((128, PADDED // 2), np.float32)
        xpc[0:64] = xt[:, :, 0, :].reshape(N_FEATURES, -1)
        xpc[64:128] = xt[:, :, 1, :].reshape(N_FEATURES, -1)
        in_maps.append({"xp": xpc.astype(ml_dtypes.bfloat16), "bm": bm,
                        "lconst": lconst, "mask": mask})
    return in_maps


def _run(data, weights, means, covariances, trace=False):
    from concourse.bass_utils import run_bass_kernel_spmd

    data = np.asarray(data, np.float32)
    bm, lconst_row, m0 = _precompute(np.asarray(weights), np.asarray(means),
                                     np.asarray(covariances))
    if "nc" not in _CACHE:
        _CACHE["nc"] = _build_nc()
    nc = _CACHE["nc"]

    in_maps = _make_inputs(data, bm, lconst_row)
    res = run_bass_kernel_spmd(nc, in_maps, list(range(N_CORES)), trace=trace)
    total = 0.0
    for cc in range(N_CORES):
        total += float(res.results[cc]["out"][0, 0]) + PER_CORE * m0
    return np.float32(total), res


def kernel(data, weights, means, covariances):
    return _run(data, weights, means, covariances)[0]


# revision 29
# speedup vs baseline: 1.1509x; 1.0063x over previous
"""GMM log-likelihood kernel for Trainium2 (Bass/Tile), 8-core data-parallel.

Math (host precompute in f64):
  B' = L^{-1} / sqrt(2),  S_k(x) = ||B'_k x||^2 = 0.5 maha-quadratic part
  wlp_k(x) = -S_k(x) + w_k . x + (C_k - m0),  w_k = B^T B mu_k,
  C_k = log pi_k - d/2 log 2pi - half_logdet_k - 0.5 ||B mu_k||^2
  out = sum_x [ m0 + log sum_k exp(wlp_k(x)) ]

Per core (25000 samples, padded to 196 tiles of 128):
  The PE runs in 64x128 row-tiled mode: even data-tiles' x^T lives in SBUF
  partitions 0:64 and computes on array rows 0:64 (tile T0), odd tiles in
  partitions 64:128 on rows 64:128 (T8) -- the two matmul streams execute
  CONCURRENTLY (the 66-row contraction only half-fills the array, and the
  HAM clock stays at 1.2 GHz for this duty cycle, so packing two matmuls
  recovers the lost 2x).  Per tile: Y psum [128,1024] (2 banks, 3 bufs)
  + a 16-col lin matmul into a per-parity group bank; a DVE STT folds
  lin+const to SBUF.  Squares split between ACT (Square -> bf16) and a
  custom DVE op sq(a)+b fusing the 64->32 fold while reading PSUM; a bf16
  TT tree (2x DVE mode) does 32->2 per (tile,k) batched per 14-tile group.
  Phase 2: exp / k-reduce / ln / mask / reduce / gpsimd partition-fold.
Host sums the 8 per-core scalars (+ m0 per real sample).
"""

import numpy as np

N_COMPONENTS = 16
N_FEATURES = 64
N_SAMPLES = 200000
N_CORES = 8
PER_CORE = N_SAMPLES // N_CORES          # 25000
TILE_P = 128
N_TILES = -(-PER_CORE // TILE_P)         # 196 (ceil)
PADDED = N_TILES * TILE_P                # 25088
KD = N_COMPONENTS * N_FEATURES           # 1024
K = N_COMPONENTS

GROUP_TILES = 14                         # tiles per tree batch
N_GROUPS = N_TILES // GROUP_TILES        # 14
PAIRS_PER_GROUP = GROUP_TILES // 2       # 7
CHUNK_PAIRS = 14                         # DMA chunk = 2 groups
N_CHUNKS = N_TILES // (2 * CHUNK_PAIRS)  # 7

# pair index -> drain path: "H" pairs split each tile's square between ACT
# (first half of each k-group) and the DVE custom op; "A" pairs let ACT
# square everything and DVE fold both tiles in one batched 2x-mode TT.
# (ACT/DVE busy-balance knob; 7/15 H measured best.  Tried and rejected:
# GPSIMD folds (~4.5us each + SBUF-port contention, +31us), 6/15 balance +
# deeper ysq/sq32 buffers + earlier lin-STTs (+2us).)
PAIR_PERIOD = 15
H_PAIR_MOD = (0, 2, 4, 6, 8, 10, 12)

_CACHE = {}


def _register_sq2():
    """Custom DVE op  out = sq(in0) + in1  (f32 internal): in0 = raw-Y half
    from PSUM (DVE may read only ONE PSUM operand), in1 = the ACT-squared
    other half from SBUF."""
    from concourse import dve_ops
    from concourse.dve_spec import Spec, Src0, Src1, sq, lower, _has_src1
    from concourse.dve_uop import DveOpSpec

    if any(op.name == "SQ1_ADD_ANT" for op in dve_ops.OPS):
        return next(op for op in dve_ops.OPS if op.name == "SQ1_ADD_ANT")
    spec = Spec(
        body=sq(Src0) + Src1,
        reference=lambda in0, in1, s0, s1, imm2: (
            in0.astype(np.float32) ** 2 + in1.astype(np.float32)),
    )
    shas = {}
    for ver in ("v3", "v4"):
        c = DveOpSpec(name="SQ1_ADD_ANT", opcode=17,
                      uops=lower(spec, ver=ver), rd1_en=_has_src1(spec))
        shas[ver] = c.sha(ver)
    op = dve_ops.DveOp("SQ1_ADD_ANT", spec, subdim=False, uops_sha=shas)
    row = max(dve_ops._SUB_OPCODE_FOR_NAME.values()) + 1
    assert row < 0x20
    dve_ops.OPS.append(op)
    dve_ops._SUB_OPCODE_FOR_NAME[op.name] = row
    dve_ops.CUSTOM_DVE_SPECS[op.name] = spec
    return op


def _build_nc():
    import concourse.tile as tile
    from concourse import bacc, mybir, bass_isa

    sq2 = _register_sq2()

    f32 = mybir.dt.float32
    bf16 = mybir.dt.bfloat16
    W = GROUP_TILES * K                  # 224 wlp columns per group
    HALF = N_TILES * TILE_P // 2         # 12544 columns of paired x^T

    nc = bacc.Bacc("TRN2", target_bir_lowering=False, debug=False,
                   num_devices=N_CORES)

    xp = nc.dram_tensor("xp", [128, HALF], bf16, kind="ExternalInput").ap()
    bm = nc.dram_tensor("bm", [128, KD + K], bf16, kind="ExternalInput").ap()
    lconst = nc.dram_tensor("lconst", [128, W], f32, kind="ExternalInput").ap()
    mask = nc.dram_tensor("mask", [128, N_TILES], f32, kind="ExternalInput").ap()
    out = nc.dram_tensor("out", [1, 1], f32, kind="ExternalOutput").ap()

    with tile.TileContext(nc) as tc:
        with (
            tc.tile_pool(name="const", bufs=1) as const_pool,
            tc.tile_pool(name="xin", bufs=2) as xin_pool,
            tc.tile_pool(name="ysq", bufs=4) as ysq_pool,
            tc.tile_pool(name="sq32", bufs=2) as sq32_pool,
            tc.tile_pool(name="tree", bufs=2) as tree_pool,
            tc.tile_pool(name="lin", bufs=2) as lin_pool,
            tc.tile_pool(name="wb", bufs=1) as wb_pool,
            tc.tile_pool(name="yp", bufs=3, space="PSUM") as yp_pool,
            tc.tile_pool(name="lpe", bufs=1, space="PSUM") as lpe_pool,
            tc.tile_pool(name="lpo", bufs=1, space="PSUM") as lpo_pool,
        ):
            bms = const_pool.tile([128, KD + K], bf16)
            nc.sync.dma_start(bms[:], bm[:])
            lcs = const_pool.tile([128, W], f32)
            msks = const_pool.tile([128, N_TILES], f32)

            wlp = wb_pool.tile([128, N_TILES * K], f32)
            ebuf = wb_pool.tile([128, N_TILES * K], f32)
            rsum = wb_pool.tile([128, N_TILES], f32)

            def drain_h(yp, sq32, sqc):
                """H path: ACT squares the 0:32 halves, DVE custom squares
                32:64 from PSUM and adds, emitting sq32[:, sqc:sqc+512]."""
                ypv = yp[:].rearrange("p (k i) -> p k i", i=64)
                ysa = ysq_pool.tile([128, 512], bf16, tag="ysa")
                nc.scalar.activation(
                    ysa[:].rearrange("p (k i) -> p k i", i=32),
                    ypv[:, :, 0:32],
                    mybir.ActivationFunctionType.Square)
                nc.vector._custom_dve(
                    sq2,
                    out=sq32[:, sqc:sqc + 512]
                    .rearrange("p (k i) -> p k i", i=32),
                    in0=ypv[:, :, 32:64],
                    in1=ysa[:].rearrange("p (k i) -> p k i", i=32),
                )

            def drain_a(ype, ypo, sq32, sqc):
                """A path: ACT squares both tiles -> one [128,2048] bf16
                buffer; one 2x-mode DVE TT folds 64->32 for the pair."""
                ysq = ysq_pool.tile([128, 2 * KD], bf16, tag="ysq")
                nc.scalar.activation(
                    ysq[:, 0:KD], ype[:], mybir.ActivationFunctionType.Square)
                nc.scalar.activation(
                    ysq[:, KD:2 * KD], ypo[:],
                    mybir.ActivationFunctionType.Square)
                yv = ysq[:].rearrange("p (k i) -> p k i", i=64)
                nc.vector.tensor_add(
                    sq32[:, sqc:sqc + 1024]
                    .rearrange("p (k i) -> p k i", i=32),
                    yv[:, :, 0:32], yv[:, :, 32:64])

            def make_tree(g, sq32, linb):
                """Deferred emitter: tree 32 -> 2 (bf16 TT @2x), STTs,
                then this group's exp + k-reduce (keeps the end tail short;
                Exp shares the ACT table set with Square, so no reloads)."""
                def emit():
                    t16 = tree_pool.tile([128, W * 16], bf16, tag="t16")
                    v = sq32[:].rearrange("p (w i) -> p w i", i=32)
                    nc.vector.tensor_add(
                        t16[:].rearrange("p (w i) -> p w i", i=16),
                        v[:, :, 0:16], v[:, :, 16:32])
                    t8 = tree_pool.tile([128, W * 8], bf16, tag="t8")
                    v = t16[:].rearrange("p (w i) -> p w i", i=16)
                    nc.vector.tensor_add(
                        t8[:].rearrange("p (w i) -> p w i", i=8),
                        v[:, :, 0:8], v[:, :, 8:16])
                    t4 = tree_pool.tile([128, W * 4], bf16, tag="t4")
                    v = t8[:].rearrange("p (w i) -> p w i", i=8)
                    nc.vector.tensor_add(
                        t4[:].rearrange("p (w i) -> p w i", i=4),
                        v[:, :, 0:4], v[:, :, 4:8])
                    t2 = tree_pool.tile([128, W * 2], bf16, tag="t2")
                    v = t4[:].rearrange("p (w i) -> p w i", i=4)
                    nc.vector.tensor_add(
                        t2[:].rearrange("p (w i) -> p w i", i=2),
                        v[:, :, 0:2], v[:, :, 2:4])
                    s1 = tree_pool.tile([128, W], f32, tag="s1")
                    v = t2[:].rearrange("p (w i) -> p w i", i=2)
                    nc.vector.scalar_tensor_tensor(
                        s1[:], v[:, :, 0:1], -1.0, v[:, :, 1:2],
                        op0=mybir.AluOpType.mult,
                        op1=mybir.AluOpType.subtract)
                    nc.vector.scalar_tensor_tensor(
                        wlp[:, g * W:(g + 1) * W], s1[:], 1.0, linb[:],
                        op0=mybir.AluOpType.mult, op1=mybir.AluOpType.add)
                    nc.scalar.activation(
                        ebuf[:, g * W:(g + 1) * W], wlp[:, g * W:(g + 1) * W],
                        mybir.ActivationFunctionType.Exp)
                    nc.vector.reduce_sum(
                        rsum[:, g * GROUP_TILES:(g + 1) * GROUP_TILES],
                        ebuf[:, g * W:(g + 1) * W]
                        .rearrange("p (t k) -> p t k", k=K),
                        axis=mybir.AxisListType.X)
                return emit

            pending_tree = None
            for c in range(N_CHUNKS):
                xb = xin_pool.tile([128, CHUNK_PAIRS * TILE_P], bf16, tag="xb")
                c0 = c * CHUNK_PAIRS * TILE_P
                nc.sync.dma_start(xb[:], xp[:, c0:c0 + CHUNK_PAIRS * TILE_P])
                if c == 0:
                    # late-needed constants AFTER the first data chunk, so the
                    # first matmuls aren't queued behind them on the DMA ring
                    nc.sync.dma_start(lcs[:], lconst[:])
                    nc.sync.dma_start(msks[:], mask[:])

                for gl in range(2):              # two 14-tile groups per chunk
                    g = 2 * c + gl
                    sq32 = sq32_pool.tile([128, GROUP_TILES * 512], bf16,
                                          tag="sq32")
                    lpe = lpe_pool.tile([128, PAIRS_PER_GROUP * K], f32,
                                        tag="lpe")
                    lpo = lpo_pool.tile([128, PAIRS_PER_GROUP * K], f32,
                                        tag="lpo")
                    for pl in range(PAIRS_PER_GROUP):
                        xc = (gl * PAIRS_PER_GROUP + pl) * TILE_P
                        ype = yp_pool.tile([128, KD], f32, tag="yp")
                        ypo = yp_pool.tile([128, KD], f32, tag="yp")
                        lhsE = xb[0:64, xc:xc + TILE_P]
                        lhsO = xb[64:128, xc:xc + TILE_P]
                        nc.tensor.matmul(ype[:, 0:512], lhsE,
                                         bms[0:64, 0:512])
                        nc.tensor.matmul(ypo[:, 0:512], lhsO,
                                         bms[64:128, 0:512])
                        nc.tensor.matmul(ype[:, 512:1024], lhsE,
                                         bms[0:64, 512:1024])
                        nc.tensor.matmul(ypo[:, 512:1024], lhsO,
                                         bms[64:128, 512:1024])
                        nc.tensor.matmul(lpe[:, pl * K:(pl + 1) * K], lhsE,
                                         bms[0:64, KD:KD + K])
                        nc.tensor.matmul(lpo[:, pl * K:(pl + 1) * K], lhsO,
                                         bms[64:128, KD:KD + K])
                        pair = g * PAIRS_PER_GROUP + pl
                        if pair % PAIR_PERIOD in H_PAIR_MOD:
                            drain_h(ype, sq32, (2 * pl) * 512)
                            drain_h(ypo, sq32, (2 * pl + 1) * 512)
                        else:
                            drain_a(ype, ypo, sq32, (2 * pl) * 512)
                        if pl == 1 and pending_tree is not None:
                            # previous group's tree, emitted mid-stream so it
                            # doesn't head-block this group's per-tile DVE ops
                            pending_tree()
                            pending_tree = None

                    # lin + const -> SBUF (frees the lp banks early)
                    linb = lin_pool.tile([128, W], f32, tag="linb")
                    lbv = linb[:].rearrange("p (t k) -> p t k", k=K)
                    lcv = lcs[:].rearrange("p (t k) -> p t k", k=K)
                    nc.vector.scalar_tensor_tensor(
                        lbv[:, 0:GROUP_TILES:2, :],
                        lpe[:].rearrange("p (t k) -> p t k", k=K),
                        1.0, lcv[:, 0:GROUP_TILES:2, :],
                        op0=mybir.AluOpType.mult, op1=mybir.AluOpType.add)
                    nc.vector.scalar_tensor_tensor(
                        lbv[:, 1:GROUP_TILES:2, :],
                        lpo[:].rearrange("p (t k) -> p t k", k=K),
                        1.0, lcv[:, 1:GROUP_TILES:2, :],
                        op0=mybir.AluOpType.mult, op1=mybir.AluOpType.add)
                    pending_tree = make_tree(g, sq32, linb)
            pending_tree()

            # phase 2 (exp/k-reduce already done per group)
            lnr = const_pool.tile([128, N_TILES], f32)
            nc.scalar.activation(lnr[:], rsum[:],
                                 mybir.ActivationFunctionType.Ln)
            msum = const_pool.tile([128, N_TILES], f32)
            nc.vector.tensor_mul(msum[:], lnr[:], msks[:])
            csum = const_pool.tile([128, 1], f32)
            nc.vector.reduce_sum(csum[:], msum[:], axis=mybir.AxisListType.X)

            res = const_pool.tile([128, 1], f32)
            nc.gpsimd.partition_all_reduce(res[:], csum[:], channels=128,
                                           reduce_op=bass_isa.ReduceOp.add)
            nc.sync.dma_start(out[:], res[0:1, :])

    nc.compile()
    return nc


def _precompute(weights, means, covariances):
    """Host-side O(K d^3) prep in float64. Returns (bm, lconst_row, m0)."""
    import ml_dtypes

    Kc, d = means.shape
    L = np.linalg.cholesky(covariances.astype(np.float64))
    half_logdet = np.log(np.diagonal(L, axis1=-2, axis2=-1)).sum(-1)
    eye = np.eye(d)
    B = np.stack([np.linalg.solve(L[k], eye) for k in range(Kc)])  # L^-1
    mu = means.astype(np.float64)
    c = np.einsum('kij,kj->ki', B, mu)                # B mu
    w_lin = np.einsum('kij,ki->kj', B, c)             # B^T B mu
    r = (c * c).sum(-1)
    C = (np.log(weights.astype(np.float64))
         - 0.5 * d * np.log(2.0 * np.pi) - half_logdet - 0.5 * r)
    m0 = float(C.max()) - 20.0
    Bs = B / np.sqrt(2.0)                             # S = 0.5 ||B x||^2

    half = np.zeros((d, KD + Kc), np.float32)
    for k in range(Kc):
        half[:, k * d:(k + 1) * d] = Bs[k].T.astype(np.float32)
    half[:, KD:] = w_lin.T.astype(np.float32)
    bm = np.vstack([half, half]).astype(ml_dtypes.bfloat16)  # [128, 1040]
    lconst_row = (C - m0).astype(np.float32)                 # [16]
    return bm, lconst_row, m0


def _make_inputs(data, bm, lconst_row):
    """8 per-core input maps: x^T parity-split into top/bottom partitions."""
    import ml_dtypes

    lconst = np.tile(lconst_row, GROUP_TILES)[None, :].repeat(128, 0)
    lconst = np.ascontiguousarray(lconst, np.float32)
    mask = np.zeros((128, N_TILES), np.float32)
    for t in range(N_TILES):
        v = min(max(PER_CORE - t * TILE_P, 0), TILE_P)
        mask[:v, t] = 1.0

    in_maps = []
    for cc in range(N_CORES):
        sl = data[cc * PER_CORE:(cc + 1) * PER_CORE]
        xt = np.zeros((N_FEATURES, PADDED), np.float32)
        xt[:, :PER_CORE] = sl.T
        xt = xt.reshape(N_FEATURES, N_TILES // 2, 2, TILE_P)
        xpc = np.empty((128, PADDED // 2), np.float32)
        xpc[0:64] = xt[:, :, 0, :].reshape(N_FEATURES, -1)
        xpc[64:128] = xt[:, :, 1, :].reshape(N_FEATURES, -1)
        in_maps.append({"xp": xpc.astype(ml_dtypes.bfloat16), "bm": bm,
                        "lconst": lconst, "mask": mask})
    return in_maps


def _run(data, weights, means, covariances, trace=False):
    from concourse.bass_utils import run_bass_kernel_spmd

    data = np.asarray(data, np.float32)
    bm, lconst_row, m0 = _precompute(np.asarray(weights), np.asarray(means),
                                     np.asarray(covariances))
    if "nc" not in _CACHE:
        _CACHE["nc"] = _build_nc()
    nc = _CACHE["nc"]

    in_maps = _make_inputs(data, bm, lconst_row)
    res = run_bass_kernel_spmd(nc, in_maps, list(range(N_CORES)), trace=trace)
    total = 0.0
    for cc in range(N_CORES):
        total += float(res.results[cc]["out"][0, 0]) + PER_CORE * m0
    return np.float32(total), res


def kernel(data, weights, means, covariances):
    return _run(data, weights, means, covariances)[0]


# revision 32
# speedup vs baseline: 1.1561x; 1.0045x over previous
"""GMM log-likelihood kernel for Trainium2 (Bass/Tile), 8-core data-parallel.

Math (host precompute in f64):
  B' = L^{-1} / sqrt(2),  S_k(x) = ||B'_k x||^2 = 0.5 maha-quadratic part
  wlp_k(x) = -S_k(x) + w_k . x + (C_k - m0),  w_k = B^T B mu_k,
  C_k = log pi_k - d/2 log 2pi - half_logdet_k - 0.5 ||B mu_k||^2
  out = sum_x [ m0 + log sum_k exp(wlp_k(x)) ]

Per core (25000 samples, padded to 196 tiles of 128):
  The PE runs in 64x128 row-tiled mode: even data-tiles' x^T lives in SBUF
  partitions 0:64 and computes on array rows 0:64 (tile T0), odd tiles in
  partitions 64:128 on rows 64:128 (T8) -- the two matmul streams execute
  CONCURRENTLY (the 66-row contraction only half-fills the array, and the
  HAM clock stays at 1.2 GHz for this duty cycle, so packing two matmuls
  recovers the lost 2x).  Per tile: Y psum [128,1024] (2 banks, 3 bufs)
  + a 16-col lin matmul into a per-parity group bank; a DVE STT folds
  lin+const to SBUF.  Squares split between ACT (Square -> bf16) and a
  custom DVE op sq(a)+b fusing the 64->32 fold while reading PSUM; a bf16
  TT tree (2x DVE mode) does 32->2 per (tile,k) batched per 14-tile group.
  Phase 2: exp / k-reduce / ln / mask / reduce / gpsimd partition-fold.
Host sums the 8 per-core scalars (+ m0 per real sample).
"""

import numpy as np

N_COMPONENTS = 16
N_FEATURES = 64
N_SAMPLES = 200000
N_CORES = 8
PER_CORE = N_SAMPLES // N_CORES          # 25000
TILE_P = 128
N_TILES = -(-PER_CORE // TILE_P)         # 196 (ceil)
PADDED = N_TILES * TILE_P                # 25088
KD = N_COMPONENTS * N_FEATURES           # 1024
K = N_COMPONENTS

GROUP_TILES = 14                         # tiles per tree batch
N_GROUPS = N_TILES // GROUP_TILES        # 14
PAIRS_PER_GROUP = GROUP_TILES // 2       # 7
CHUNK_PAIRS = 14                         # DMA chunk = 2 groups
N_CHUNKS = N_TILES // (2 * CHUNK_PAIRS)  # 7

# pair index -> drain path: "H" pairs split each tile's square between ACT
# (first half of each k-group) and the DVE custom op; "A" pairs let ACT
# square everything and DVE fold both tiles in one batched 2x-mode TT.
# (ACT/DVE busy-balance knob; 7/15 H measured best.  Tried and rejected:
# GPSIMD folds (~4.5us each + SBUF-port contention, +31us), 6/15 balance +
# deeper ysq/sq32 buffers + earlier lin-STTs (+2us).)
PAIR_PERIOD = 15
H_PAIR_MOD = (0, 2, 4, 6, 8, 10, 12)

_CACHE = {}


def _register_sq2():
    """Custom DVE op  out = sq(in0) + in1  (f32 internal): in0 = raw-Y half
    from PSUM (DVE may read only ONE PSUM operand), in1 = the ACT-squared
    other half from SBUF."""
    from concourse import dve_ops
    from concourse.dve_spec import Spec, Src0, Src1, sq, lower, _has_src1
    from concourse.dve_uop import DveOpSpec

    if any(op.name == "SQ1_ADD_ANT" for op in dve_ops.OPS):
        return next(op for op in dve_ops.OPS if op.name == "SQ1_ADD_ANT")
    spec = Spec(
        body=sq(Src0) + Src1,
        reference=lambda in0, in1, s0, s1, imm2: (
            in0.astype(np.float32) ** 2 + in1.astype(np.float32)),
    )
    shas = {}
    for ver in ("v3", "v4"):
        c = DveOpSpec(name="SQ1_ADD_ANT", opcode=17,
                      uops=lower(spec, ver=ver), rd1_en=_has_src1(spec))
        shas[ver] = c.sha(ver)
    op = dve_ops.DveOp("SQ1_ADD_ANT", spec, subdim=False, uops_sha=shas)
    row = max(dve_ops._SUB_OPCODE_FOR_NAME.values()) + 1
    assert row < 0x20
    dve_ops.OPS.append(op)
    dve_ops._SUB_OPCODE_FOR_NAME[op.name] = row
    dve_ops.CUSTOM_DVE_SPECS[op.name] = spec
    return op


def _build_nc():
    import concourse.tile as tile
    from concourse import bacc, mybir, bass_isa

    sq2 = _register_sq2()

    f32 = mybir.dt.float32
    bf16 = mybir.dt.bfloat16
    W = GROUP_TILES * K                  # 224 wlp columns per group
    HALF = N_TILES * TILE_P // 2         # 12544 columns of paired x^T

    nc = bacc.Bacc("TRN2", target_bir_lowering=False, debug=False,
                   num_devices=N_CORES)

    xp = nc.dram_tensor("xp", [128, HALF], bf16, kind="ExternalInput").ap()
    bm = nc.dram_tensor("bm", [128, KD + K], bf16, kind="ExternalInput").ap()
    lconst = nc.dram_tensor("lconst", [128, W], f32, kind="ExternalInput").ap()
    mask = nc.dram_tensor("mask", [128, N_TILES], f32, kind="ExternalInput").ap()
    out = nc.dram_tensor("out", [1, 1], f32, kind="ExternalOutput").ap()

    with tile.TileContext(nc) as tc:
        with (
            tc.tile_pool(name="const", bufs=1) as const_pool,
            tc.tile_pool(name="xin", bufs=2) as xin_pool,
            tc.tile_pool(name="ysq", bufs=4) as ysq_pool,
            tc.tile_pool(name="sq32", bufs=2) as sq32_pool,
            tc.tile_pool(name="tree", bufs=2) as tree_pool,
            tc.tile_pool(name="lin", bufs=2) as lin_pool,
            tc.tile_pool(name="wb", bufs=1) as wb_pool,
            tc.tile_pool(name="yp", bufs=3, space="PSUM") as yp_pool,
            tc.tile_pool(name="lpe", bufs=1, space="PSUM") as lpe_pool,
            tc.tile_pool(name="lpo", bufs=1, space="PSUM") as lpo_pool,
        ):
            bms = const_pool.tile([128, KD + K], bf16)
            nc.sync.dma_start(bms[:], bm[:])
            lcs = const_pool.tile([128, W], f32)
            msks = const_pool.tile([128, N_TILES], f32)

            wlp = wb_pool.tile([128, N_TILES * K], f32)
            ebuf = wb_pool.tile([128, N_TILES * K], f32)
            rsum = wb_pool.tile([128, N_TILES], f32)

            def drain_h(yp, sq32, sqc):
                """H path: ACT squares the 0:32 halves, DVE custom squares
                32:64 from PSUM and adds, emitting sq32[:, sqc:sqc+512]."""
                ypv = yp[:].rearrange("p (k i) -> p k i", i=64)
                ysa = ysq_pool.tile([128, 512], bf16, tag="ysa")
                nc.scalar.activation(
                    ysa[:].rearrange("p (k i) -> p k i", i=32),
                    ypv[:, :, 0:32],
                    mybir.ActivationFunctionType.Square)
                nc.vector._custom_dve(
                    sq2,
                    out=sq32[:, sqc:sqc + 512]
                    .rearrange("p (k i) -> p k i", i=32),
                    in0=ypv[:, :, 32:64],
                    in1=ysa[:].rearrange("p (k i) -> p k i", i=32),
                )

            def drain_a(ype, ypo, sq32, sqc):
                """A path: ACT squares both tiles -> one [128,2048] bf16
                buffer; one 2x-mode DVE TT folds 64->32 for the pair."""
                ysq = ysq_pool.tile([128, 2 * KD], bf16, tag="ysq")
                nc.scalar.activation(
                    ysq[:, 0:KD], ype[:], mybir.ActivationFunctionType.Square)
                nc.scalar.activation(
                    ysq[:, KD:2 * KD], ypo[:],
                    mybir.ActivationFunctionType.Square)
                yv = ysq[:].rearrange("p (k i) -> p k i", i=64)
                nc.vector.tensor_add(
                    sq32[:, sqc:sqc + 1024]
                    .rearrange("p (k i) -> p k i", i=32),
                    yv[:, :, 0:32], yv[:, :, 32:64])

            def tree_tt(sq32, w0, w1):
                """TT levels 32 -> 2 for wlp columns [w0, w1); returns t2."""
                n = w1 - w0
                t16 = tree_pool.tile([128, n * 16], bf16, tag="t16")
                v = sq32[:, w0 * 32:w1 * 32].rearrange("p (w i) -> p w i",
                                                       i=32)
                nc.vector.tensor_add(
                    t16[:].rearrange("p (w i) -> p w i", i=16),
                    v[:, :, 0:16], v[:, :, 16:32])
                t8 = tree_pool.tile([128, n * 8], bf16, tag="t8")
                v = t16[:].rearrange("p (w i) -> p w i", i=16)
                nc.vector.tensor_add(
                    t8[:].rearrange("p (w i) -> p w i", i=8),
                    v[:, :, 0:8], v[:, :, 8:16])
                t4 = tree_pool.tile([128, n * 4], bf16, tag="t4")
                v = t8[:].rearrange("p (w i) -> p w i", i=8)
                nc.vector.tensor_add(
                    t4[:].rearrange("p (w i) -> p w i", i=4),
                    v[:, :, 0:4], v[:, :, 4:8])
                t2 = tree_pool.tile([128, n * 2], bf16, tag="t2")
                v = t4[:].rearrange("p (w i) -> p w i", i=4)
                nc.vector.tensor_add(
                    t2[:].rearrange("p (w i) -> p w i", i=2),
                    v[:, :, 0:2], v[:, :, 2:4])
                return t2

            def tree_fin(g, t2, linb, w0, w1):
                """Finish [w0, w1): wlp = -(a+b) + lin+const."""
                n = w1 - w0
                s1 = tree_pool.tile([128, n], f32, tag="s1")
                v = t2[:].rearrange("p (w i) -> p w i", i=2)
                nc.vector.scalar_tensor_tensor(
                    s1[:], v[:, :, 0:1], -1.0, v[:, :, 1:2],
                    op0=mybir.AluOpType.mult, op1=mybir.AluOpType.subtract)
                nc.vector.scalar_tensor_tensor(
                    wlp[:, g * W + w0:g * W + w1], s1[:], 1.0,
                    linb[:, w0:w1],
                    op0=mybir.AluOpType.mult, op1=mybir.AluOpType.add)

            def make_tree(g, sq32, linb):
                """Deferred emitter: tree 32 -> 2 (bf16 TT @2x), STTs,
                then this group's exp + k-reduce (keeps the end tail short;
                Exp shares the ACT table set with Square, so no reloads)."""
                def emit():
                    t16 = tree_pool.tile([128, W * 16], bf16, tag="t16")
                    v = sq32[:].rearrange("p (w i) -> p w i", i=32)
                    nc.vector.tensor_add(
                        t16[:].rearrange("p (w i) -> p w i", i=16),
                        v[:, :, 0:16], v[:, :, 16:32])
                    t8 = tree_pool.tile([128, W * 8], bf16, tag="t8")
                    v = t16[:].rearrange("p (w i) -> p w i", i=16)
                    nc.vector.tensor_add(
                        t8[:].rearrange("p (w i) -> p w i", i=8),
                        v[:, :, 0:8], v[:, :, 8:16])
                    t4 = tree_pool.tile([128, W * 4], bf16, tag="t4")
                    v = t8[:].rearrange("p (w i) -> p w i", i=8)
                    nc.vector.tensor_add(
                        t4[:].rearrange("p (w i) -> p w i", i=4),
                        v[:, :, 0:4], v[:, :, 4:8])
                    t2 = tree_pool.tile([128, W * 2], bf16, tag="t2")
                    v = t4[:].rearrange("p (w i) -> p w i", i=4)
                    nc.vector.tensor_add(
                        t2[:].rearrange("p (w i) -> p w i", i=2),
                        v[:, :, 0:2], v[:, :, 2:4])
                    s1 = tree_pool.tile([128, W], f32, tag="s1")
                    v = t2[:].rearrange("p (w i) -> p w i", i=2)
                    nc.vector.scalar_tensor_tensor(
                        s1[:], v[:, :, 0:1], -1.0, v[:, :, 1:2],
                        op0=mybir.AluOpType.mult,
                        op1=mybir.AluOpType.subtract)
                    nc.vector.scalar_tensor_tensor(
                        wlp[:, g * W:(g + 1) * W], s1[:], 1.0, linb[:],
                        op0=mybir.AluOpType.mult, op1=mybir.AluOpType.add)
                    nc.scalar.activation(
                        ebuf[:, g * W:(g + 1) * W], wlp[:, g * W:(g + 1) * W],
                        mybir.ActivationFunctionType.Exp)
                    nc.vector.reduce_sum(
                        rsum[:, g * GROUP_TILES:(g + 1) * GROUP_TILES],
                        ebuf[:, g * W:(g + 1) * W]
                        .rearrange("p (t k) -> p t k", k=K),
                        axis=mybir.AxisListType.X)
                return emit

            pending_tree = None
            for c in range(N_CHUNKS):
                xb = xin_pool.tile([128, CHUNK_PAIRS * TILE_P], bf16, tag="xb")
                c0 = c * CHUNK_PAIRS * TILE_P
                nc.sync.dma_start(xb[:], xp[:, c0:c0 + CHUNK_PAIRS * TILE_P])
                if c == 0:
                    # late-needed constants AFTER the first data chunk, so the
                    # first matmuls aren't queued behind them on the DMA ring
                    nc.sync.dma_start(lcs[:], lconst[:])
                    nc.sync.dma_start(msks[:], mask[:])

                for gl in range(2):              # two 14-tile groups per chunk
                    g = 2 * c + gl
                    sq32 = sq32_pool.tile([128, GROUP_TILES * 512], bf16,
                                          tag="sq32")
                    lpe = lpe_pool.tile([128, PAIRS_PER_GROUP * K], f32,
                                        tag="lpe")
                    lpo = lpo_pool.tile([128, PAIRS_PER_GROUP * K], f32,
                                        tag="lpo")
                    for pl in range(PAIRS_PER_GROUP):
                        xc = (gl * PAIRS_PER_GROUP + pl) * TILE_P
                        ype = yp_pool.tile([128, KD], f32, tag="yp")
                        ypo = yp_pool.tile([128, KD], f32, tag="yp")
                        lhsE = xb[0:64, xc:xc + TILE_P]
                        lhsO = xb[64:128, xc:xc + TILE_P]
                        nc.tensor.matmul(ype[:, 0:512], lhsE,
                                         bms[0:64, 0:512])
                        nc.tensor.matmul(ypo[:, 0:512], lhsO,
                                         bms[64:128, 0:512])
                        nc.tensor.matmul(ype[:, 512:1024], lhsE,
                                         bms[0:64, 512:1024])
                        nc.tensor.matmul(ypo[:, 512:1024], lhsO,
                                         bms[64:128, 512:1024])
                        nc.tensor.matmul(lpe[:, pl * K:(pl + 1) * K], lhsE,
                                         bms[0:64, KD:KD + K])
                        nc.tensor.matmul(lpo[:, pl * K:(pl + 1) * K], lhsO,
                                         bms[64:128, KD:KD + K])
                        pair = g * PAIRS_PER_GROUP + pl
                        if pair % PAIR_PERIOD in H_PAIR_MOD:
                            drain_h(ype, sq32, (2 * pl) * 512)
                            drain_h(ypo, sq32, (2 * pl + 1) * 512)
                        else:
                            drain_a(ype, ypo, sq32, (2 * pl) * 512)
                        if pl == 1 and pending_tree is not None:
                            # previous group's tree, emitted mid-stream so it
                            # doesn't head-block this group's per-tile DVE ops
                            pending_tree()
                            pending_tree = None
                        if g == N_GROUPS - 1 and pl == 5:
                            # last group: fold tiles 0:8 early so only half a
                            # tree remains serialized after the final matmul
                            t2a = tree_tt(sq32, 0, 8 * K)

                    # lin + const -> SBUF (frees the lp banks early)
                    linb = lin_pool.tile([128, W], f32, tag="linb")
                    lbv = linb[:].rearrange("p (t k) -> p t k", k=K)
                    lcv = lcs[:].rearrange("p (t k) -> p t k", k=K)
                    nc.vector.scalar_tensor_tensor(
                        lbv[:, 0:GROUP_TILES:2, :],
                        lpe[:].rearrange("p (t k) -> p t k", k=K),
                        1.0, lcv[:, 0:GROUP_TILES:2, :],
                        op0=mybir.AluOpType.mult, op1=mybir.AluOpType.add)
                    nc.vector.scalar_tensor_tensor(
                        lbv[:, 1:GROUP_TILES:2, :],
                        lpo[:].rearrange("p (t k) -> p t k", k=K),
                        1.0, lcv[:, 1:GROUP_TILES:2, :],
                        op0=mybir.AluOpType.mult, op1=mybir.AluOpType.add)
                    if g < N_GROUPS - 1:
                        pending_tree = make_tree(g, sq32, linb)
                    else:
                        # finish the last group inline: half-B tree + both
                        # halves' wlp, then one exp + k-reduce for the group
                        t2b = tree_tt(sq32, 8 * K, W)
                        tree_fin(g, t2a, linb, 0, 8 * K)
                        tree_fin(g, t2b, linb, 8 * K, W)
                        nc.scalar.activation(
                            ebuf[:, g * W:(g + 1) * W],
                            wlp[:, g * W:(g + 1) * W],
                            mybir.ActivationFunctionType.Exp)
                        nc.vector.reduce_sum(
                            rsum[:, g * GROUP_TILES:(g + 1) * GROUP_TILES],
                            ebuf[:, g * W:(g + 1) * W]
                            .rearrange("p (t k) -> p t k", k=K),
                            axis=mybir.AxisListType.X)

            # phase 2 (exp/k-reduce already done per group)
            lnr = const_pool.tile([128, N_TILES], f32)
            nc.scalar.activation(lnr[:], rsum[:],
                                 mybir.ActivationFunctionType.Ln)
            msum = const_pool.tile([128, N_TILES], f32)
            nc.vector.tensor_mul(msum[:], lnr[:], msks[:])
            csum = const_pool.tile([128, 1], f32)
            nc.vector.reduce_sum(csum[:], msum[:], axis=mybir.AxisListType.X)

            res = const_pool.tile([128, 1], f32)
            nc.gpsimd.partition_all_reduce(res[:], csum[:], channels=128,
                                           reduce_op=bass_isa.ReduceOp.add)
            nc.sync.dma_start(out[:], res[0:1, :])

    nc.compile()
    return nc


def _precompute(weights, means, covariances):
    """Host-side O(K d^3) prep in float64. Returns (bm, lconst_row, m0)."""
    import ml_dtypes

    Kc, d = means.shape
    L = np.linalg.cholesky(covariances.astype(np.float64))
    half_logdet = np.log(np.diagonal(L, axis1=-2, axis2=-1)).sum(-1)
    eye = np.eye(d)
    B = np.stack([np.linalg.solve(L[k], eye) for k in range(Kc)])  # L^-1
    mu = means.astype(np.float64)
    c = np.einsum('kij,kj->ki', B, mu)                # B mu
    w_lin = np.einsum('kij,ki->kj', B, c)             # B^T B mu
    r = (c * c).sum(-1)
    C = (np.log(weights.astype(np.float64))
         - 0.5 * d * np.log(2.0 * np.pi) - half_logdet - 0.5 * r)
    m0 = float(C.max()) - 20.0
    Bs = B / np.sqrt(2.0)                             # S = 0.5 ||B x||^2

    half = np.zeros((d, KD + Kc), np.float32)
    for k in range(Kc):
        half[:, k * d:(k + 1) * d] = Bs[k].T.astype(np.float32)
    half[:, KD:] = w_lin.T.astype(np.float32)
    bm = np.vstack([half, half]).astype(ml_dtypes.bfloat16)  # [128, 1040]
    lconst_row = (C - m0).astype(np.float32)                 # [16]
    return bm, lconst_row, m0


def _make_inputs(data, bm, lconst_row):
    """8 per-core input maps: x^T parity-split into top/bottom partitions."""
    import ml_dtypes

    lconst = np.tile(lconst_row, GROUP_TILES)[None, :].repeat(128, 0)
    lconst = np.ascontiguousarray(lconst, np.float32)
    mask = np.zeros((128, N_TILES), np.float32)
    for t in range(N_TILES):
        v = min(max(PER_CORE - t * TILE_P, 0), TILE_P)
        mask[:v, t] = 1.0

    in_maps = []
    for cc in range(N_CORES):
        sl = data[cc * PER_CORE:(cc + 1) * PER_CORE]
        xt = np.zeros((N_FEATURES, PADDED), np.float32)
        xt[:, :PER_CORE] = sl.T
        xt = xt.reshape(N_FEATURES, N_TILES // 2, 2, TILE_P)
        xpc = np.empty((128, PADDED // 2), np.float32)
        xpc[0:64] = xt[:, :, 0, :].reshape(N_FEATURES, -1)
        xpc[64:128] = xt[:, :, 1, :].reshape(N_FEATURES, -1)
        in_maps.append({"xp": xpc.astype(ml_dtypes.bfloat16), "bm": bm,
                        "lconst": lconst, "mask": mask})
    return in_maps


def _run(data, weights, means, covariances, trace=False):
    from concourse.bass_utils import run_bass_kernel_spmd

    data = np.asarray(data, np.float32)
    bm, lconst_row, m0 = _precompute(np.asarray(weights), np.asarray(means),
                                     np.asarray(covariances))
    if "nc" not in _CACHE:
        _CACHE["nc"] = _build_nc()
    nc = _CACHE["nc"]

    in_maps = _make_inputs(data, bm, lconst_row)
    res = run_bass_kernel_spmd(nc, in_maps, list(range(N_CORES)), trace=trace)
    total = 0.0
    for cc in range(N_CORES):
        total += float(res.results[cc]["out"][0, 0]) + PER_CORE * m0
    return np.float32(total), res


def kernel(data, weights, means, covariances):
    return _run(data, weights, means, covariances)[0]
